# revision 1
# baseline (speedup 1.0000x reference)
"""Modulated deformable conv (DFConv2d) Trainium2 Bass kernel.

Problem (hardcoded): x [4,256,64,64] f32; w_off [27,256,3,3]; b_off [27];
w_conv [256,256,3,3]; out [4,256,64,64].  K=3, pad=1, stride=1, dil=1.

Sharding: 8 cores = batch(4) x spatial-half(2).  Each core computes
out[b, :, s*32:(s+1)*32, :] (2048 output positions).

Per-core pipeline:
  1. PE: offset conv -> om[27, 2048] (accumulated over 18 shifted matmuls).
  2. PE transpose om -> position-major layout [128, 16*27]; small DVE/ACT
     pipeline computes bilinear corner weights (validity + mask folded in)
     and gather row indices (int32) of the NHWC image.
  3. gpsimd indirect-DMA gathers x_nhwc rows (x-corner pairs: 2 positions x
     256 ch per index) -> [128 pos, 2 slots, 256 ch].
  4. DVE fused scalar_tensor_tensor MACs combine the 4 corners (weights are
     per-partition scalars) -> sampled [128 pos, 256 ch] per tap.
  5. PE transposes sampled into [ck, pos]; big matmul w2[2304,256] x
     sampled[2304, 512-chunk] accumulating in PSUM; DMA out.
"""

import numpy as np

import concourse.bass as bass
import concourse.bacc as bacc
import concourse.tile as tile
from concourse import mybir
from concourse.bass_utils import run_bass_kernel_spmd

F32 = mybir.dt.float32
BF16 = mybir.dt.bfloat16
F32R = mybir.dt.float32r
I32 = mybir.dt.int32
AF = mybir.ActivationFunctionType
OP = mybir.AluOpType

B, C, H, W, O = 4, 256, 64, 64, 256
K2 = 9
HW = H * W                # 4096
POS = 2048                # positions per core (32 rows)
NCH = POS // 128          # 16 chunks of 128 positions
MAGIC = 12582912.0        # 1.5*2^23 float-floor magic (ulp=1 for |v|<2^22)


def build_program(debug=False, hw_gather_walk=True, reps=1):
    nc = bacc.Bacc("TRN2", target_bir_lowering=False)

    xs_t = nc.dram_tensor("xs", (C, 34 * 66), F32R, kind="ExternalInput")
    xn_t = nc.dram_tensor("xn", (HW, 4 * C), BF16, kind="ExternalInput")
    wof_t = nc.dram_tensor("wof", (C, K2 * 27), F32R, kind="ExternalInput")
    bof_t = nc.dram_tensor("bof", (27, 1), F32, kind="ExternalInput")
    w2_t = nc.dram_tensor("w2", (K2 * C, O), F32R, kind="ExternalInput")
    byt_t = nc.dram_tensor("byt", (128, NCH * K2), F32, kind="ExternalInput")
    bxt_t = nc.dram_tensor("bxt", (128, NCH * K2), F32, kind="ExternalInput")
    idn_t = nc.dram_tensor("idn", (128, 128), F32, kind="ExternalInput")
    idnb_t = nc.dram_tensor("idnb", (128, 128), BF16, kind="ExternalInput")
    out_t = nc.dram_tensor("out", (O, POS), F32, kind="ExternalOutput")

    with tile.TileContext(nc) as tc:
        with (
            tc.tile_pool(name="const", bufs=1) as constp,
            tc.tile_pool(name="small", bufs=1) as smallp,
            tc.tile_pool(name="gbuf", bufs=3) as gbufp,
            tc.tile_pool(name="acc", bufs=4) as accp,
            tc.tile_pool(name="samp", bufs=1) as sampp,
            tc.tile_pool(name="ps_om", bufs=2, space="PSUM") as ps_om,
            tc.tile_pool(name="ps_tp", bufs=2, space="PSUM") as ps_tp,
            tc.tile_pool(name="ps_out", bufs=1, space="PSUM") as ps_out,
        ):
            # ---- load constants ----
            xs_sb = []
            wof_sb = []
            for ct in range(2):
                t = constp.tile([128, 34 * 66], F32R, tag=f"xs{ct}", name=f"xs{ct}")
                nc.sync.dma_start(out=t[:], in_=xs_t[ct * 128:(ct + 1) * 128, :])
                xs_sb.append(t)
                t = constp.tile([128, K2 * 27], F32R, tag=f"wof{ct}", name=f"wof{ct}")
                nc.sync.dma_start(out=t[:], in_=wof_t[ct * 128:(ct + 1) * 128, :])
                wof_sb.append(t)
            w2_sb = []
            for kt in range(18):
                t = constp.tile([128, O], F32R, tag=f"w2_{kt}", name=f"w2_{kt}")
                nc.sync.dma_start(out=t[:], in_=w2_t[kt * 128:(kt + 1) * 128, :])
                w2_sb.append(t)
            byt = constp.tile([128, NCH * K2], F32, tag="byt", name="byt")
            nc.sync.dma_start(out=byt[:], in_=byt_t[:])
            bxt = constp.tile([128, NCH * K2], F32, tag="bxt", name="bxt")
            nc.sync.dma_start(out=bxt[:], in_=bxt_t[:])
            idn = constp.tile([128, 128], F32, tag="idn", name="idn")
            nc.sync.dma_start(out=idn[:], in_=idn_t[:])
            idnb = constp.tile([128, 128], BF16, tag="idnb", name="idnb")
            nc.sync.dma_start(out=idnb[:], in_=idnb_t[:])
            bof = constp.tile([27, 1], F32, tag="bof", name="bof")
            nc.sync.dma_start(out=bof[:], in_=bof_t[:])

            for rep in range(reps):
                # ---- offset conv: om[27, 2048] ----
                om = smallp.tile([27, POS], F32, tag="om", name="om")
                for ch in range(4):  # 512-position chunks (8 output rows)
                    pom = ps_om.tile([27, 512], F32, tag="pom", name="pom")
                    first = True
                    for k in range(K2):
                        ki, kj = k // 3, k % 3
                        for ct in range(2):
                            rhs = (xs_sb[ct][:].rearrange("p (r c) -> p r c", r=34)
                                   [:, ch * 8 + ki: ch * 8 + ki + 8, kj: kj + 64])
                            nc.tensor.matmul(
                                pom[:],
                                lhsT=wof_sb[ct][:, k * 27:(k + 1) * 27],
                                rhs=rhs,
                                start=first,
                                stop=(k == K2 - 1 and ct == 1),
                            )
                            first = False
                    nc.scalar.activation(
                        out=om[:, ch * 512:(ch + 1) * 512], in_=pom[:],
                        func=AF.Identity, bias=bof[:, 0:1], scale=1.0,
                    )

                # ---- transpose om to position-major: omt[128, 16*27] ----
                omt = smallp.tile([128, NCH * 27], F32, tag="omt", name="omt")
                for ch in range(NCH):
                    ptp = ps_tp.tile([128, 27], F32, tag="omtp", name="omtp")
                    nc.tensor.transpose(
                        out=ptp[:],
                        in_=om[:, ch * 128:(ch + 1) * 128],
                        identity=idn[:27, :27],
                    )
                    nc.vector.tensor_copy(
                        out=omt[:, ch * 27:(ch + 1) * 27], in_=ptp[:])

                omt_r = omt[:].rearrange("p (ch c) -> p ch c", c=27)

                def sm(tag):
                    return smallp.tile([128, NCH * K2], F32, tag=tag, name=tag)

                # mask = sigmoid(logits)
                maskt = sm("maskt")
                nc.scalar.activation(
                    out=maskt[:].rearrange("p (ch k) -> p ch k", k=K2),
                    in_=omt_r[:, :, 18:27], func=AF.Sigmoid)

                # sample coords
                def v3(ap):
                    return ap.rearrange("p (ch k) -> p ch k", k=K2)

                ys = sm("ys")
                nc.vector.tensor_tensor(
                    out=v3(ys[:]), in0=omt_r[:, :, 0:18:2],
                    in1=v3(byt[:]), op=OP.add)
                xs = sm("xs_")
                nc.vector.tensor_tensor(
                    out=v3(xs[:]), in0=omt_r[:, :, 1:18:2],
                    in1=v3(bxt[:]), op=OP.add)

                def floorf(v, tagp):
                    # two separate instructions so the +2^23 result is rounded
                    # to f32 in SBUF before subtracting (fused ts would keep
                    # full precision and defeat the magic-number rounding)
                    r = sm(tagp + "_r")
                    nc.vector.tensor_scalar(
                        out=r[:], in0=v[:], scalar1=MAGIC, scalar2=None,
                        op0=OP.add)
                    nc.vector.tensor_scalar(
                        out=r[:], in0=r[:], scalar1=MAGIC, scalar2=None,
                        op0=OP.subtract)
                    corr = sm(tagp + "_c")
                    nc.vector.tensor_tensor(out=corr[:], in0=r[:], in1=v[:],
                                            op=OP.is_gt)
                    f = sm(tagp + "_f")
                    nc.vector.tensor_tensor(out=f[:], in0=r[:], in1=corr[:],
                                            op=OP.subtract)
                    frac = sm(tagp + "_fr")
                    nc.vector.tensor_tensor(out=frac[:], in0=v[:], in1=f[:],
                                            op=OP.subtract)
                    return f, frac

                y0f, ly = floorf(ys, "fy")
                x0f, lx = floorf(xs, "fx")

                # y corner weights (validity folded)
                wy0 = sm("wy0")
                nc.vector.tensor_scalar(out=wy0[:], in0=ly[:], scalar1=-1.0,
                                        scalar2=1.0, op0=OP.mult, op1=OP.add)
                y0c = sm("y0c")
                nc.vector.tensor_scalar(out=y0c[:], in0=y0f[:], scalar1=0.0,
                                        scalar2=63.0, op0=OP.max, op1=OP.min)
                v0 = sm("v0")
                nc.vector.tensor_tensor(out=v0[:], in0=y0c[:], in1=y0f[:],
                                        op=OP.is_equal)
                nc.vector.tensor_tensor(out=wy0[:], in0=wy0[:], in1=v0[:],
                                        op=OP.mult)
                y1f = sm("y1f")
                nc.vector.tensor_scalar(out=y1f[:], in0=y0f[:], scalar1=1.0,
                                        scalar2=None, op0=OP.add)
                y1c = sm("y1c")
                nc.vector.tensor_scalar(out=y1c[:], in0=y1f[:], scalar1=0.0,
                                        scalar2=63.0, op0=OP.max, op1=OP.min)
                v1 = sm("v1")
                nc.vector.tensor_tensor(out=v1[:], in0=y1c[:], in1=y1f[:],
                                        op=OP.is_equal)
                wy1 = sm("wy1")
                nc.vector.tensor_tensor(out=wy1[:], in0=ly[:], in1=v1[:],
                                        op=OP.mult)

                # x: clamp base to [0,62]; 2-wide slot weights with edge remap
                x0c = sm("x0c")
                nc.vector.tensor_scalar(out=x0c[:], in0=x0f[:], scalar1=0.0,
                                        scalar2=62.0, op0=OP.max, op1=OP.min)
                i0 = sm("i0")
                nc.vector.tensor_tensor(out=i0[:], in0=x0f[:], in1=x0c[:],
                                        op=OP.is_equal)
                en = sm("en")
                nc.vector.tensor_scalar(out=en[:], in0=x0f[:], scalar1=-1.0,
                                        scalar2=None, op0=OP.is_equal)
                e63 = sm("e63")
                nc.vector.tensor_scalar(out=e63[:], in0=x0f[:], scalar1=63.0,
                                        scalar2=None, op0=OP.is_equal)
                wx0 = sm("wx0")
                nc.vector.tensor_scalar(out=wx0[:], in0=lx[:], scalar1=-1.0,
                                        scalar2=1.0, op0=OP.mult, op1=OP.add)
                ta = sm("ta")
                tb = sm("tb")
                wp0 = sm("wp0")
                nc.vector.tensor_tensor(out=ta[:], in0=wx0[:], in1=i0[:], op=OP.mult)
                nc.vector.tensor_tensor(out=tb[:], in0=lx[:], in1=en[:], op=OP.mult)
                nc.vector.tensor_tensor(out=wp0[:], in0=ta[:], in1=tb[:], op=OP.add)
                tc2 = sm("tc2")
                td = sm("td")
                wp1 = sm("wp1")
                nc.vector.tensor_tensor(out=tc2[:], in0=lx[:], in1=i0[:], op=OP.mult)
                nc.vector.tensor_tensor(out=td[:], in0=wx0[:], in1=e63[:], op=OP.mult)
                nc.vector.tensor_tensor(out=wp1[:], in0=tc2[:], in1=td[:], op=OP.add)
                # fold mask
                nc.vector.tensor_tensor(out=wp0[:], in0=wp0[:], in1=maskt[:],
                                        op=OP.mult)
                nc.vector.tensor_tensor(out=wp1[:], in0=wp1[:], in1=maskt[:],
                                        op=OP.mult)

                # y-slot remap (mirror of x): slot a holds row y0c+a
                iy0 = sm("iy0")
                nc.vector.tensor_tensor(out=iy0[:], in0=y0c[:], in1=y0f[:],
                                        op=OP.is_equal)
                eyn = sm("eyn")
                nc.vector.tensor_scalar(out=eyn[:], in0=y0f[:], scalar1=-1.0,
                                        scalar2=None, op0=OP.is_equal)
                ta2 = sm("ta2")
                tb2 = sm("tb2")
                wq0 = sm("wq0")
                nc.vector.tensor_tensor(out=ta2[:], in0=wy0[:], in1=iy0[:],
                                        op=OP.mult)
                nc.vector.tensor_tensor(out=tb2[:], in0=wy1[:], in1=eyn[:],
                                        op=OP.mult)
                nc.vector.tensor_tensor(out=wq0[:], in0=ta2[:], in1=tb2[:],
                                        op=OP.add)
                wq1 = sm("wq1")
                nc.vector.tensor_tensor(out=wq1[:], in0=wy1[:], in1=iy0[:],
                                        op=OP.mult)
                # final corner weights
                cw = {}
                for a, wya in ((0, wq0), (1, wq1)):
                    for bslot, wpb in ((0, wp0), (1, wp1)):
                        t = sm(f"cw{a}{bslot}")
                        nc.vector.tensor_tensor(out=t[:], in0=wya[:], in1=wpb[:],
                                                op=OP.mult)
                        cw[(a, bslot)] = t

                if debug:
                    dbg = {}
                    for nm, t in [("ys", ys), ("xs_", xs), ("y0f", y0f),
                                  ("x0f", x0f), ("ly", ly), ("lx", lx),
                                  ("maskt", maskt), ("wy0", wy0), ("wy1", wy1),
                                  ("wp0", wp0), ("wp1", wp1), ("y0c", y0c),
                                  ("y1c", y1c), ("x0c", x0c)]:
                        dt = nc.dram_tensor(f"dbg_{nm}", (128, NCH * K2), F32,
                                            kind="ExternalOutput")
                        nc.sync.dma_start(out=dt[:], in_=t[:])
                    for (a, bs), t in cw.items():
                        dt = nc.dram_tensor(f"dbg_cw{a}{bs}", (128, NCH * K2),
                                            F32, kind="ExternalOutput")
                        nc.sync.dma_start(out=dt[:], in_=t[:])
                    dom = nc.dram_tensor("dbg_om", (27, POS), F32,
                                         kind="ExternalOutput")
                    nc.sync.dma_start(out=dom[:], in_=om[:])

                # gather index: patch-table row y0c*64 + x0c, one per
                # position; layout [p,(k,cq,c4)] -> each call's offset is a
                # contiguous [128,1] slice
                idx_both = smallp.tile([128, NCH * K2], I32, tag="idx_both", name="idx_both")
                tfi = sm("idxf")
                nc.vector.tensor_scalar(out=tfi[:], in0=y0c[:], scalar1=64.0,
                                        scalar2=None, op0=OP.mult)
                nc.vector.tensor_tensor(out=tfi[:], in0=tfi[:], in1=x0c[:],
                                        op=OP.add)
                nc.vector.tensor_copy(
                    out=idx_both[:].rearrange("p (k cq c4) -> p k cq c4",
                                              k=K2, cq=4),
                    in_=tfi[:].rearrange("p (cq c4 k) -> p k cq c4", k=K2,
                                         cq=4))

                # ---- gather + MAC + transpose + matmul, per 512-pos chunk ----
                for cq in range(4):
                    samp = {}
                    for tap in range(K2):
                        # one gather: both y-corners x 4 idx-chunks x (2pos*256c)
                        g = gbufp.tile([128, 4, 1024], BF16, tag="g", name="g")
                        base = tap * 16 + cq * 4
                        for c4 in range(4):
                            offs = idx_both[:, base + c4: base + c4 + 1]
                            nc.gpsimd.indirect_dma_start(
                                out=g[:, c4, :], out_offset=None, in_=xn_t[:],
                                in_offset=bass.IndirectOffsetOnAxis(
                                    ap=offs, axis=0),
                            )
                        acc = accp.tile([128, 4, 256], BF16, tag=f"acc{tap % 2}", name=f"acc{tap % 2}")
                        for s4 in range(4):
                            ch = cq * 4 + s4
                            first = True
                            for a in range(2):
                                for bslot in range(2):
                                    scal = (cw[(a, bslot)][:]
                                            [:, ch * K2 + tap: ch * K2 + tap + 1])
                                    gin = g[:, s4,
                                            (a * 2 + bslot) * 256:
                                            (a * 2 + bslot + 1) * 256]
                                    if first:
                                        nc.scalar.activation(
                                            out=acc[:, s4, :], in_=gin,
                                            func=AF.Copy, scale=scal)
                                        first = False
                                    else:
                                        nc.vector.scalar_tensor_tensor(
                                            out=acc[:, s4, :], in0=gin,
                                            scalar=scal, in1=acc[:, s4, :],
                                            op0=OP.mult, op1=OP.add)
                        # transpose to [c, pos]
                        for ct in range(2):
                            ptp = ps_tp.tile([128, 4, 128], BF16, tag="stp", name="stp")
                            for s4 in range(4):
                                nc.tensor.transpose(
                                    out=ptp[:, s4, :],
                                    in_=acc[:, s4, ct * 128:(ct + 1) * 128],
                                    identity=idnb[:],
                                )
                            st = sampp.tile([128, 512], F32R,
                                            tag=f"samp{tap * 2 + ct}", name=f"samp{tap * 2 + ct}")
                            nc.scalar.activation(
                                out=st[:],
                                in_=ptp[:].rearrange("p a b -> p (a b)"),
                                func=AF.Copy)
                            samp[tap * 2 + ct] = st

                    for m in range(2):
                        po = ps_out.tile([128, 512], F32, tag=f"po{m}", name=f"po{m}")
                        for kt in range(18):
                            nc.tensor.matmul(
                                po[:],
                                lhsT=w2_sb[kt][:, m * 128:(m + 1) * 128],
                                rhs=samp[kt][:],
                                start=(kt == 0),
                                stop=(kt == 17),
                            )
                        osb = accp.tile([128, 512], F32, tag=f"osb{m}",
                                        name=f"osb{m}")
                        nc.scalar.activation(out=osb[:], in_=po[:], func=AF.Copy)
                        nc.sync.dma_start(
                            out=out_t[m * 128:(m + 1) * 128,
                                      cq * 512:(cq + 1) * 512],
                            in_=osb[:],
                        )

    nc.compile()
    return nc


def host_inputs(x, w_off, b_off, w_conv):
    """Build the 8 per-core input maps (numpy only, layout prep)."""
    import ml_dtypes
    x = np.asarray(x, np.float32)
    w_off = np.asarray(w_off, np.float32)
    b_off = np.asarray(b_off, np.float32)
    w_conv = np.asarray(w_conv, np.float32)

    xp = np.zeros((B, C, 66, 66), np.float32)
    xp[:, :, 1:65, 1:65] = x
    wof = np.ascontiguousarray(
        w_off.reshape(27, C, K2).transpose(1, 2, 0)).reshape(C, K2 * 27)
    w2 = np.ascontiguousarray(
        w_conv.reshape(O, C, K2).transpose(2, 1, 0)).reshape(K2 * C, O)
    bof = b_off.reshape(27, 1).astype(np.float32)
    idn = np.eye(128, dtype=np.float32)

    in_maps = []
    for core in range(8):
        b, s = core // 2, core % 2
        xs = np.ascontiguousarray(
            xp[b, :, s * 32: s * 32 + 34, :]).reshape(C, 34 * 66)
        xn = np.ascontiguousarray(x[b].transpose(1, 2, 0)).reshape(HW, C)
        xnp = np.zeros((HW + 65, C), np.float32)
        xnp[:HW] = xn
        xn_bf = np.concatenate(
            [xnp[0:HW], xnp[1:HW + 1], xnp[64:HW + 64], xnp[65:HW + 65]],
            axis=1).astype(ml_dtypes.bfloat16)
        p = np.arange(128)
        ch = np.arange(NCH)
        k = np.arange(K2)
        pos = ch[None, :, None] * 128 + p[:, None, None]  # [128, 16, 1]
        byt = (s * 32 + pos // 64 + (k[None, None, :] // 3) - 1).astype(
            np.float32).reshape(128, NCH * K2)
        bxt = (pos % 64 + (k[None, None, :] % 3) - 1).astype(
            np.float32).reshape(128, NCH * K2)
        in_maps.append({
            "xs": xs, "xn": xn_bf, "wof": wof, "bof": bof, "w2": w2,
            "byt": byt, "bxt": bxt, "idn": idn,
            "idnb": idn.astype(ml_dtypes.bfloat16),
        })
    return in_maps


_NC = None


def kernel(x, w_off, b_off, w_conv):
    global _NC
    if _NC is None:
        _NC = build_program()
    in_maps = host_inputs(x, w_off, b_off, w_conv)
    res = run_bass_kernel_spmd(_NC, in_maps, core_ids=list(range(8)))
    out = np.empty((B, O, H, W), np.float32)
    for core in range(8):
        b, s = core // 2, core % 2
        out[b, :, s * 32:(s + 1) * 32, :] = res.results[core]["out"].reshape(
            O, 32, 64)
    return out



# revision 3
# speedup vs baseline: 1.0724x; 1.0724x over previous
"""Modulated deformable conv (DFConv2d) Trainium2 Bass kernel, v3.

Problem (hardcoded): x [4,256,64,64] f32; w_off [27,256,3,3]; b_off [27];
w_conv [256,256,3,3]; out [4,256,64,64].  K=3, pad=1, stride=1, dil=1.

Sharding: 8 cores = batch(4) x spatial-half(2).  Each core computes
out[b, :, s*32:(s+1)*32, :] (2048 output positions), pipelined in 4
chunks (cq) of 512 positions.

v3 design (vs v1 baseline at 220us):
  * The gather is bound by SWDGE descriptor-gen on gpsimd: 994ns fixed per
    indirect_dma_start call, one offset per partition (multi-offset calls
    are not supported by the deployed ucode - verified empirically), so
    2048 pos x 9 taps / 128 = 144 calls = ~150us of Pool time is the hard
    floor.  v3 makes Pool a pure descriptor-gen engine running at ~100%
    for the whole kernel and fits everything else underneath:
  * int8 gather table on a zero-padded 66x66 grid.  Halves gather DMA
    bytes (103us -> 52us) and removes all validity/edge-remap vector ops:
    out-of-bounds corners read genuine zeros from the pad; positions whose
    corner window leaves the padded grid have an exactly-zero true sample
    and are killed via one validity multiply folded into the mask (which
    also absorbs the int8 dequant scale).
  * The 576 corner-MAC ops ([128pos, 256ch], per-partition scalar weights;
    no DVE fast mode exists for ptr-scalar STT) are split DVE/ACT:
    corner00 on ACT (copy*scale init); corner01 mostly as ACT-mult +
    DVE-tensor-add pairs; corners 10/11 as DVE STT.  Sample-stage
    PSUM->SBUF copies on ACT.
  * Whole per-chunk chain (offset conv -> coords -> 36 gathers -> MAC ->
    transpose -> big matmul) is software-pipelined via tile pools, so the
    former 45us serial prologue overlaps the steady state.
"""

import numpy as np

import concourse.bass as bass
import concourse.bacc as bacc
import concourse.tile as tile
from concourse import mybir
from concourse.bass_utils import run_bass_kernel_spmd

F32 = mybir.dt.float32
F32R = mybir.dt.float32r
BF16 = mybir.dt.bfloat16
I32 = mybir.dt.int32
I8 = mybir.dt.int8
AF = mybir.ActivationFunctionType
OP = mybir.AluOpType

B, C, H, W, O = 4, 256, 64, 64, 256
K2 = 9
POS = 2048                 # positions per core (32 output rows)
NCQ = 4                    # position chunks per core
CQP = POS // NCQ           # 512 positions per chunk
NT = 66                    # padded grid side (64 + 2 pad)
TROWS = NT * NT            # 4356 gather-table rows
MAGIC = 12582912.0         # 1.5*2^23 float-floor magic


def build_program(debug=False, hw_gather_walk=True, reps=1):
    nc = bacc.Bacc("TRN2", target_bir_lowering=False)

    xs_t = nc.dram_tensor("xs", (C, 34 * 66), F32R, kind="ExternalInput")
    wof_t = nc.dram_tensor("wof", (C, K2 * 27), F32R, kind="ExternalInput")
    bof_t = nc.dram_tensor("bof", (27, 1), F32, kind="ExternalInput")
    w2_t = nc.dram_tensor("w2", (K2 * C, O), F32R, kind="ExternalInput")
    xq_t = nc.dram_tensor("xq", (TROWS, 4 * C), I8, kind="ExternalInput")
    byt_t = nc.dram_tensor("byt", (128, NCQ * 36), F32, kind="ExternalInput")
    bxt_t = nc.dram_tensor("bxt", (128, NCQ * 36), F32, kind="ExternalInput")
    ss_t = nc.dram_tensor("ss", (128, 1), F32, kind="ExternalInput")
    idnb_t = nc.dram_tensor("idnb", (128, 128), BF16, kind="ExternalInput")
    idn27_t = nc.dram_tensor("idn27", (27, 27), F32, kind="ExternalInput")
    out_t = nc.dram_tensor("out", (O, POS), F32, kind="ExternalOutput")

    with tile.TileContext(nc) as tc:
        with (
            tc.tile_pool(name="const", bufs=1) as constp,
            tc.tile_pool(name="coord", bufs=2) as coordp,
            tc.tile_pool(name="gbuf", bufs=8) as gbufp,
            tc.tile_pool(name="tmpb", bufs=4) as tmpp,
            tc.tile_pool(name="accb", bufs=3) as accp,
            tc.tile_pool(name="samp", bufs=2) as sampp,
            tc.tile_pool(name="outb", bufs=2) as outp,
            tc.tile_pool(name="ps_om", bufs=1, space="PSUM") as ps_om,
            tc.tile_pool(name="ps_t27", bufs=2, space="PSUM") as ps_t27,
            tc.tile_pool(name="ps_s", bufs=3, space="PSUM") as ps_s,
            tc.tile_pool(name="ps_out", bufs=2, space="PSUM") as ps_out,
        ):
            # ---- load constants ----
            xs_sb = []
            wof_sb = []
            for ct in range(2):
                t = constp.tile([128, 34 * 66], F32R, tag=f"xs{ct}", name=f"xs{ct}")
                nc.sync.dma_start(out=t[:], in_=xs_t[ct * 128:(ct + 1) * 128, :])
                xs_sb.append(t)
                t = constp.tile([128, K2 * 27], F32R, tag=f"wof{ct}", name=f"wof{ct}")
                nc.sync.dma_start(out=t[:], in_=wof_t[ct * 128:(ct + 1) * 128, :])
                wof_sb.append(t)
            w2_sb = []
            for kt in range(18):
                t = constp.tile([128, O], F32R, tag=f"w2_{kt}", name=f"w2_{kt}")
                nc.sync.dma_start(out=t[:], in_=w2_t[kt * 128:(kt + 1) * 128, :])
                w2_sb.append(t)
            byt = constp.tile([128, NCQ * 36], F32, tag="byt", name="byt")
            nc.sync.dma_start(out=byt[:], in_=byt_t[:])
            bxt = constp.tile([128, NCQ * 36], F32, tag="bxt", name="bxt")
            nc.sync.dma_start(out=bxt[:], in_=bxt_t[:])
            ss = constp.tile([128, 1], F32, tag="ss", name="ss")
            nc.sync.dma_start(out=ss[:], in_=ss_t[:])
            idnb = constp.tile([128, 128], BF16, tag="idnb", name="idnb")
            nc.sync.dma_start(out=idnb[:], in_=idnb_t[:])
            idn27 = constp.tile([27, 27], F32, tag="idn27", name="idn27")
            nc.sync.dma_start(out=idn27[:], in_=idn27_t[:])
            bof = constp.tile([27, 1], F32, tag="bof", name="bof")
            nc.sync.dma_start(out=bof[:], in_=bof_t[:])

            for rep in range(reps):
                for cq in range(NCQ):
                    # ---- offset conv: om[27, 512] for this chunk ----
                    pom = ps_om.tile([27, CQP], F32, tag="pom", name="pom")
                    first = True
                    for k in range(K2):
                        ki, kj = k // 3, k % 3
                        for ct in range(2):
                            rhs = (xs_sb[ct][:]
                                   .rearrange("p (r c) -> p r c", r=34)
                                   [:, cq * 8 + ki: cq * 8 + ki + 8, kj: kj + 64])
                            nc.tensor.matmul(
                                pom[:],
                                lhsT=wof_sb[ct][:, k * 27:(k + 1) * 27],
                                rhs=rhs,
                                start=first,
                                stop=(k == K2 - 1 and ct == 1),
                            )
                            first = False
                    om = coordp.tile([27, CQP], F32, tag="om", name="om")
                    nc.scalar.activation(out=om[:], in_=pom[:],
                                         func=AF.Identity, bias=bof[:, 0:1],
                                         scale=1.0)

                    # ---- transpose to position-major omt[128, c4, 27] ----
                    omt = coordp.tile([128, 4, 27], F32, tag="omt", name="omt")
                    for c4 in range(4):
                        ptp = ps_t27.tile([128, 27], F32, tag="omtp", name="omtp")
                        nc.tensor.transpose(
                            out=ptp[:],
                            in_=om[:, c4 * 128:(c4 + 1) * 128],
                            identity=idn27[:],
                        )
                        nc.vector.tensor_copy(out=omt[:, c4, :], in_=ptp[:])

                    def sm(tag, dt=F32):
                        return coordp.tile([128, 36], dt, tag=tag, name=tag)

                    def v3(ap):
                        return ap.rearrange("p (k c4) -> p k c4", k=K2)

                    # ---- coords (all [128, 36], free = (k, c4)) ----
                    ys = sm("ys")
                    nc.vector.tensor_tensor(
                        out=v3(ys[:]), in0=omt[:, :, 0:18:2].rearrange(
                            "p c4 k -> p k c4"),
                        in1=v3(byt[:, cq * 36:(cq + 1) * 36]), op=OP.add)
                    xs_ = sm("xs_")
                    nc.vector.tensor_tensor(
                        out=v3(xs_[:]), in0=omt[:, :, 1:18:2].rearrange(
                            "p c4 k -> p k c4"),
                        in1=v3(bxt[:, cq * 36:(cq + 1) * 36]), op=OP.add)

                    def floorf(v, tagp):
                        r = sm(tagp + "_r")
                        nc.vector.tensor_scalar(out=r[:], in0=v[:],
                                                scalar1=MAGIC, scalar2=None,
                                                op0=OP.add)
                        nc.vector.tensor_scalar(out=r[:], in0=r[:],
                                                scalar1=MAGIC, scalar2=None,
                                                op0=OP.subtract)
                        corr = sm(tagp + "_c")
                        nc.vector.tensor_tensor(out=corr[:], in0=r[:],
                                                in1=v[:], op=OP.is_gt)
                        f = sm(tagp + "_f")
                        nc.vector.tensor_tensor(out=f[:], in0=r[:],
                                                in1=corr[:], op=OP.subtract)
                        frac = sm(tagp + "_fr")
                        nc.vector.tensor_tensor(out=frac[:], in0=v[:],
                                                in1=f[:], op=OP.subtract)
                        return f, frac

                    py, ly = floorf(ys, "fy")   # py = floor(y)+1 (pad baked)
                    px, lx = floorf(xs_, "fx")

                    pyc = sm("pyc")
                    nc.vector.tensor_scalar(out=pyc[:], in0=py[:], scalar1=0.0,
                                            scalar2=64.0, op0=OP.max, op1=OP.min)
                    pxc = sm("pxc")
                    nc.vector.tensor_scalar(out=pxc[:], in0=px[:], scalar1=0.0,
                                            scalar2=64.0, op0=OP.max, op1=OP.min)
                    vy = sm("vy")
                    nc.vector.tensor_tensor(out=vy[:], in0=pyc[:], in1=py[:],
                                            op=OP.is_equal)
                    vx = sm("vx")
                    nc.vector.tensor_tensor(out=vx[:], in0=pxc[:], in1=px[:],
                                            op=OP.is_equal)
                    vv = sm("vv")
                    nc.vector.tensor_tensor(out=vv[:], in0=vy[:], in1=vx[:],
                                            op=OP.mult)

                    # mask = sigmoid(logits) * validity * dequant scale
                    mk = sm("mk")
                    nc.scalar.activation(
                        out=v3(mk[:]),
                        in_=omt[:, :, 18:27].rearrange("p c4 k -> p k c4"),
                        func=AF.Sigmoid)
                    mv = sm("mv")
                    nc.vector.tensor_tensor(out=mv[:], in0=mk[:], in1=vv[:],
                                            op=OP.mult)
                    nc.vector.scalar_tensor_tensor(
                        out=mv[:], in0=mv[:], scalar=ss[:, 0:1], in1=mv[:],
                        op0=OP.mult, op1=OP.bypass)

                    # corner weights: cw[a][b] = wy_a * wx_b * mv
                    wx1m = sm("wx1m")
                    nc.vector.tensor_tensor(out=wx1m[:], in0=lx[:], in1=mv[:],
                                            op=OP.mult)
                    wx0m = sm("wx0m")
                    nc.vector.tensor_tensor(out=wx0m[:], in0=mv[:], in1=wx1m[:],
                                            op=OP.subtract)
                    wy0 = sm("wy0")
                    nc.vector.tensor_scalar(out=wy0[:], in0=ly[:], scalar1=-1.0,
                                            scalar2=1.0, op0=OP.mult, op1=OP.add)
                    cw = {}
                    for (a, wya) in ((0, wy0), (1, ly)):
                        for (b, wxb) in ((0, wx0m), (1, wx1m)):
                            t = sm(f"cw{a}{b}")
                            nc.vector.tensor_tensor(out=t[:], in0=wya[:],
                                                    in1=wxb[:], op=OP.mult)
                            cw[(a, b)] = t

                    # ---- gather index: row = pyc*66 + pxc (int32) ----
                    tfi = sm("tfi")
                    nc.vector.tensor_scalar(out=tfi[:], in0=pyc[:],
                                            scalar1=float(NT), scalar2=None,
                                            op0=OP.mult)
                    nc.vector.tensor_tensor(out=tfi[:], in0=tfi[:], in1=pxc[:],
                                            op=OP.add)
                    idxi = coordp.tile([128, 36], I32, tag="idxi", name="idxi")
                    nc.vector.tensor_copy(out=idxi[:], in_=tfi[:])

                    if debug:
                        for nm, t in [("ys", ys), ("xs_", xs_), ("py", py),
                                      ("px", px), ("ly", ly), ("lx", lx),
                                      ("mv", mv), ("tfi", tfi)]:
                            dt_ = nc.dram_tensor(f"dbg_{nm}_{cq}", (128, 36),
                                                 F32, kind="ExternalOutput")
                            nc.sync.dma_start(out=dt_[:], in_=t[:])
                        for (a, b), t in cw.items():
                            dt_ = nc.dram_tensor(f"dbg_cw{a}{b}_{cq}",
                                                 (128, 36), F32,
                                                 kind="ExternalOutput")
                            nc.sync.dma_start(out=dt_[:], in_=t[:])

                    # ---- per tap: 4 gathers + corner MAC + transpose ----
                    samp = {}
                    for k in range(K2):
                        acc = accp.tile([128, 4, C], BF16, tag="acc",
                                        name="acc")
                        for c4 in range(4):
                            col = k * 4 + c4
                            g = gbufp.tile([128, 4 * C], I8, tag="g", name="g")
                            nc.gpsimd.indirect_dma_start(
                                out=g[:], out_offset=None, in_=xq_t[:],
                                in_offset=bass.IndirectOffsetOnAxis(
                                    ap=idxi[:, col:col + 1], axis=0),
                            )
                            gs = [g[:, j * C:(j + 1) * C] for j in range(4)]
                            # corner order in row: (y0x0),(y0x1),(y1x0),(y1x1)
                            nc.scalar.activation(
                                out=acc[:, c4, :], in_=gs[0], func=AF.Copy,
                                scale=cw[(0, 0)][:, col:col + 1])
                            if col % 3 != 0:
                                # corner01 as ACT-mult + DVE add (load balance)
                                tmp = tmpp.tile([128, C], BF16, tag="tmp",
                                                name="tmp")
                                nc.scalar.activation(
                                    out=tmp[:], in_=gs[1], func=AF.Copy,
                                    scale=cw[(0, 1)][:, col:col + 1])
                                nc.vector.tensor_tensor(
                                    out=acc[:, c4, :], in0=acc[:, c4, :],
                                    in1=tmp[:], op=OP.add)
                            else:
                                nc.vector.scalar_tensor_tensor(
                                    out=acc[:, c4, :], in0=gs[1],
                                    scalar=cw[(0, 1)][:, col:col + 1],
                                    in1=acc[:, c4, :], op0=OP.mult, op1=OP.add)
                            nc.vector.scalar_tensor_tensor(
                                out=acc[:, c4, :], in0=gs[2],
                                scalar=cw[(1, 0)][:, col:col + 1],
                                in1=acc[:, c4, :], op0=OP.mult, op1=OP.add)
                            nc.vector.scalar_tensor_tensor(
                                out=acc[:, c4, :], in0=gs[3],
                                scalar=cw[(1, 1)][:, col:col + 1],
                                in1=acc[:, c4, :], op0=OP.mult, op1=OP.add)
                        # transpose [pos, ch] -> [ch, pos]
                        ps = ps_s.tile([128, 2, 4, 128], BF16, tag="ps",
                                       name="ps")
                        for ct in range(2):
                            for c4 in range(4):
                                nc.tensor.transpose(
                                    out=ps[:, ct, c4, :],
                                    in_=acc[:, c4, ct * 128:(ct + 1) * 128],
                                    identity=idnb[:],
                                )
                        st = sampp.tile([128, 2, 4, 128], BF16,
                                        tag=f"samp{k}", name=f"samp{k}")
                        nc.scalar.activation(
                            out=st[:].rearrange("p a b c -> p (a b c)"),
                            in_=ps[:].rearrange("p a b c -> p (a b c)"),
                            func=AF.Copy)
                        samp[k] = st

                    # ---- big matmul: out[256, 512] for this chunk ----
                    for m in range(2):
                        po = ps_out.tile([128, CQP], F32, tag="po", name="po")
                        for k in range(K2):
                            for ct in range(2):
                                nc.tensor.matmul(
                                    po[:],
                                    lhsT=w2_sb[k * 2 + ct][:, m * 128:
                                                           (m + 1) * 128],
                                    rhs=samp[k][:, ct].rearrange(
                                        "p a b -> p (a b)"),
                                    start=(k == 0 and ct == 0),
                                    stop=(k == K2 - 1 and ct == 1),
                                )
                        osb = outp.tile([128, CQP], F32, tag="osb", name="osb")
                        nc.scalar.activation(out=osb[:], in_=po[:],
                                             func=AF.Copy)
                        nc.sync.dma_start(
                            out=out_t[m * 128:(m + 1) * 128,
                                      cq * CQP:(cq + 1) * CQP],
                            in_=osb[:],
                        )

    nc.compile()
    return nc


def host_inputs(x, w_off, b_off, w_conv):
    """Build the 8 per-core input maps (numpy only, layout prep)."""
    import ml_dtypes
    x = np.asarray(x, np.float32)
    w_off = np.asarray(w_off, np.float32)
    b_off = np.asarray(b_off, np.float32)
    w_conv = np.asarray(w_conv, np.float32)

    xp = np.zeros((B, C, 66, 66), np.float32)
    xp[:, :, 1:65, 1:65] = x
    wof = np.ascontiguousarray(
        w_off.reshape(27, C, K2).transpose(1, 2, 0)).reshape(C, K2 * 27)
    w2 = np.ascontiguousarray(
        w_conv.reshape(O, C, K2).transpose(2, 1, 0)).reshape(K2 * C, O)
    bof = b_off.reshape(27, 1).astype(np.float32)
    idnb = np.eye(128, dtype=ml_dtypes.bfloat16)
    idn27 = np.eye(27, dtype=np.float32)

    # int8 gather tables, one per batch image, on a 67x67 construction pad
    xq_b = []
    ss_b = []
    for b in range(B):
        s = float(np.abs(x[b]).max()) / 127.0
        q67 = np.zeros((67, 67, C), np.int8)
        q67[1:65, 1:65] = np.clip(
            np.rint(x[b].transpose(1, 2, 0) / s), -127, 127).astype(np.int8)
        xq = np.concatenate(
            [q67[:66, :66, None], q67[:66, 1:67, None],
             q67[1:67, :66, None], q67[1:67, 1:67, None]],
            axis=2).reshape(TROWS, 4 * C)
        xq_b.append(np.ascontiguousarray(xq))
        ss_b.append(np.full((128, 1), s, np.float32))

    in_maps = []
    p = np.arange(128)
    k = np.arange(K2)
    c4 = np.arange(4)
    cqv = np.arange(NCQ)
    for core in range(8):
        b, sh = core // 2, core % 2
        xs = np.ascontiguousarray(
            xp[b, :, sh * 32: sh * 32 + 34, :]).reshape(C, 34 * 66)
        # pos = cq*512 + c4*128 + p ; row = pos//64 ; col = pos%64
        pos = (cqv[:, None, None, None] * 512 + c4[None, None, :, None] * 128
               + p[None, None, None, :])                    # [cq, 1, c4, p]
        pos = np.broadcast_to(pos, (NCQ, K2, 4, 128))
        row = sh * 32 + pos // 64
        colw = pos % 64
        # padded-grid base incl. +1 pad offset: floor(y)+1 = floor(y + base+1)
        byt = (row + (k[None, :, None, None] // 3)).astype(np.float32)
        bxt = (colw + (k[None, :, None, None] % 3)).astype(np.float32)
        byt = byt.transpose(3, 0, 1, 2).reshape(128, NCQ * 36)
        bxt = bxt.transpose(3, 0, 1, 2).reshape(128, NCQ * 36)
        in_maps.append({
            "xs": xs, "wof": wof, "bof": bof, "w2": w2, "xq": xq_b[b],
            "byt": np.ascontiguousarray(byt),
            "bxt": np.ascontiguousarray(bxt),
            "ss": ss_b[b], "idnb": idnb, "idn27": idn27,
        })
    return in_maps


_NC = None


def kernel(x, w_off, b_off, w_conv):
    global _NC
    if _NC is None:
        _NC = build_program()
    in_maps = host_inputs(x, w_off, b_off, w_conv)
    res = run_bass_kernel_spmd(_NC, in_maps, core_ids=list(range(8)))
    out = np.empty((B, O, H, W), np.float32)
    for core in range(8):
        b, sh = core // 2, core % 2
        out[b, :, sh * 32:(sh + 1) * 32, :] = res.results[core]["out"].reshape(
            O, 32, 64)
    return out


# revision 6
# speedup vs baseline: 1.2144x; 1.1324x over previous
"""Modulated deformable conv (DFConv2d) Trainium2 Bass kernel, v3.

Problem (hardcoded): x [4,256,64,64] f32; w_off [27,256,3,3]; b_off [27];
w_conv [256,256,3,3]; out [4,256,64,64].  K=3, pad=1, stride=1, dil=1.

Sharding: 8 cores = batch(4) x spatial-half(2).  Each core computes
out[b, :, s*32:(s+1)*32, :] (2048 output positions), pipelined in 4
chunks (cq) of 512 positions.

v3 design (vs v1 baseline at 220us):
  * The gather is bound by SWDGE descriptor-gen on gpsimd: 994ns fixed per
    indirect_dma_start call, one offset per partition (multi-offset calls
    are not supported by the deployed ucode - verified empirically), so
    2048 pos x 9 taps / 128 = 144 calls = ~150us of Pool time is the hard
    floor.  v3 makes Pool a pure descriptor-gen engine running at ~100%
    for the whole kernel and fits everything else underneath:
  * int8 gather table on a zero-padded 66x66 grid.  Halves gather DMA
    bytes (103us -> 52us) and removes all validity/edge-remap vector ops:
    out-of-bounds corners read genuine zeros from the pad; positions whose
    corner window leaves the padded grid have an exactly-zero true sample
    and are killed via one validity multiply folded into the mask (which
    also absorbs the int8 dequant scale).
  * The 576 corner-MAC ops ([128pos, 256ch], per-partition scalar weights;
    no DVE fast mode exists for ptr-scalar STT) are split DVE/ACT:
    corner00 on ACT (copy*scale init); corner01 mostly as ACT-mult +
    DVE-tensor-add pairs; corners 10/11 as DVE STT.  Sample-stage
    PSUM->SBUF copies on ACT.
  * Whole per-chunk chain (offset conv -> coords -> 36 gathers -> MAC ->
    transpose -> big matmul) is software-pipelined via tile pools, so the
    former 45us serial prologue overlaps the steady state.
"""

import numpy as np

import concourse.bass as bass
import concourse.bacc as bacc
import concourse.tile as tile
from concourse import mybir
from concourse.bass_utils import run_bass_kernel_spmd

F32 = mybir.dt.float32
F32R = mybir.dt.float32r
BF16 = mybir.dt.bfloat16
I32 = mybir.dt.int32
I8 = mybir.dt.int8
AF = mybir.ActivationFunctionType
OP = mybir.AluOpType

B, C, H, W, O = 4, 256, 64, 64, 256
K2 = 9
POS = 2048                 # positions per core (32 output rows)
NCQ = 4                    # position chunks per core
CQP = POS // NCQ           # 512 positions per chunk
NT = 66                    # padded grid side (64 + 2 pad)
TROWS = NT * NT            # 4356 gather-table rows
MAGIC = 12582912.0         # 1.5*2^23 float-floor magic


def build_program(debug=False, hw_gather_walk=True, reps=1):
    nc = bacc.Bacc("TRN2", target_bir_lowering=False)

    xs_t = nc.dram_tensor("xs", (C, 34 * 66), F32R, kind="ExternalInput")
    wof_t = nc.dram_tensor("wof", (C, K2 * 27), F32R, kind="ExternalInput")
    bof_t = nc.dram_tensor("bof", (27, 1), F32, kind="ExternalInput")
    w2_t = nc.dram_tensor("w2", (K2 * C, O), F32R, kind="ExternalInput")
    xq_t = nc.dram_tensor("xq", (TROWS, 4 * C), I8, kind="ExternalInput")
    byt_t = nc.dram_tensor("byt", (128, NCQ * 36), F32, kind="ExternalInput")
    bxt_t = nc.dram_tensor("bxt", (128, NCQ * 36), F32, kind="ExternalInput")
    ss_t = nc.dram_tensor("ss", (128, 1), F32, kind="ExternalInput")
    idnb_t = nc.dram_tensor("idnb", (128, 128), BF16, kind="ExternalInput")
    idn27_t = nc.dram_tensor("idn27", (27, 27), F32, kind="ExternalInput")
    out_t = nc.dram_tensor("out", (O, POS), F32, kind="ExternalOutput")

    with tile.TileContext(nc) as tc:
        with (
            tc.tile_pool(name="const", bufs=1) as constp,
            tc.tile_pool(name="coord", bufs=2) as coordp,
            tc.tile_pool(name="gbuf", bufs=8) as gbufp,
            tc.tile_pool(name="tmpb", bufs=4) as tmpp,
            tc.tile_pool(name="accb", bufs=3) as accp,
            tc.tile_pool(name="samp", bufs=2) as sampp,
            tc.tile_pool(name="outb", bufs=2) as outp,
            tc.tile_pool(name="ps_om", bufs=1, space="PSUM") as ps_om,
            tc.tile_pool(name="ps_t27", bufs=2, space="PSUM") as ps_t27,
            tc.tile_pool(name="ps_s", bufs=3, space="PSUM") as ps_s,
            tc.tile_pool(name="ps_out", bufs=2, space="PSUM") as ps_out,
        ):
            # ---- load constants (prologue-critical first, w2 last) ----
            xs_sb = []
            wof_sb = []
            for ct in range(2):
                t = constp.tile([128, 34 * 66], F32R, tag=f"xs{ct}", name=f"xs{ct}")
                nc.sync.dma_start(out=t[:], in_=xs_t[ct * 128:(ct + 1) * 128, :])
                xs_sb.append(t)
                t = constp.tile([128, K2 * 27], F32R, tag=f"wof{ct}", name=f"wof{ct}")
                nc.sync.dma_start(out=t[:], in_=wof_t[ct * 128:(ct + 1) * 128, :])
                wof_sb.append(t)
            byt = constp.tile([128, NCQ * 36], F32, tag="byt", name="byt")
            nc.sync.dma_start(out=byt[:], in_=byt_t[:])
            bxt = constp.tile([128, NCQ * 36], F32, tag="bxt", name="bxt")
            nc.sync.dma_start(out=bxt[:], in_=bxt_t[:])
            ss = constp.tile([128, 1], F32, tag="ss", name="ss")
            nc.sync.dma_start(out=ss[:], in_=ss_t[:])
            idn27 = constp.tile([27, 27], F32, tag="idn27", name="idn27")
            nc.sync.dma_start(out=idn27[:], in_=idn27_t[:])
            bof = constp.tile([27, 1], F32, tag="bof", name="bof")
            nc.sync.dma_start(out=bof[:], in_=bof_t[:])
            idnb = constp.tile([128, 128], BF16, tag="idnb", name="idnb")
            nc.sync.dma_start(out=idnb[:], in_=idnb_t[:])
            w2_sb = []
            for kt in range(18):
                t = constp.tile([128, O], F32R, tag=f"w2_{kt}", name=f"w2_{kt}")
                nc.sync.dma_start(out=t[:], in_=w2_t[kt * 128:(kt + 1) * 128, :])
                w2_sb.append(t)

            def head(cq):
                    # ---- offset conv: om[27, 512] for this chunk ----
                    pom = ps_om.tile([27, CQP], F32, tag="pom", name="pom")
                    first = True
                    for k in range(K2):
                        ki, kj = k // 3, k % 3
                        for ct in range(2):
                            rhs = (xs_sb[ct][:]
                                   .rearrange("p (r c) -> p r c", r=34)
                                   [:, cq * 8 + ki: cq * 8 + ki + 8, kj: kj + 64])
                            nc.tensor.matmul(
                                pom[:],
                                lhsT=wof_sb[ct][:, k * 27:(k + 1) * 27],
                                rhs=rhs,
                                start=first,
                                stop=(k == K2 - 1 and ct == 1),
                            )
                            first = False
                    om = coordp.tile([27, CQP], F32, tag="om", name="om")
                    nc.scalar.activation(out=om[:], in_=pom[:],
                                         func=AF.Identity, bias=bof[:, 0:1],
                                         scale=1.0)

                    # ---- transpose to position-major omt[128, c4, 27] ----
                    omt = coordp.tile([128, 4, 27], F32, tag="omt", name="omt")
                    for c4 in range(4):
                        ptp = ps_t27.tile([128, 27], F32, tag="omtp", name="omtp")
                        nc.tensor.transpose(
                            out=ptp[:],
                            in_=om[:, c4 * 128:(c4 + 1) * 128],
                            identity=idn27[:],
                        )
                        nc.vector.tensor_copy(out=omt[:, c4, :], in_=ptp[:])

                    def sm(tag, dt=F32):
                        return coordp.tile([128, 36], dt, tag=tag, name=tag)

                    def v3(ap):
                        return ap.rearrange("p (k c4) -> p k c4", k=K2)

                    # ---- coords (all [128, 36], free = (k, c4)) ----
                    ys = sm("ys")
                    nc.vector.tensor_tensor(
                        out=v3(ys[:]), in0=omt[:, :, 0:18:2].rearrange(
                            "p c4 k -> p k c4"),
                        in1=v3(byt[:, cq * 36:(cq + 1) * 36]), op=OP.add)
                    xs_ = sm("xs_")
                    nc.vector.tensor_tensor(
                        out=v3(xs_[:]), in0=omt[:, :, 1:18:2].rearrange(
                            "p c4 k -> p k c4"),
                        in1=v3(bxt[:, cq * 36:(cq + 1) * 36]), op=OP.add)

                    def floorf(v, tagp):
                        r = sm(tagp + "_r")
                        nc.vector.tensor_scalar(out=r[:], in0=v[:],
                                                scalar1=MAGIC, scalar2=None,
                                                op0=OP.add)
                        nc.vector.tensor_scalar(out=r[:], in0=r[:],
                                                scalar1=MAGIC, scalar2=None,
                                                op0=OP.subtract)
                        corr = sm(tagp + "_c")
                        nc.vector.tensor_tensor(out=corr[:], in0=r[:],
                                                in1=v[:], op=OP.is_gt)
                        f = sm(tagp + "_f")
                        nc.vector.tensor_tensor(out=f[:], in0=r[:],
                                                in1=corr[:], op=OP.subtract)
                        frac = sm(tagp + "_fr")
                        nc.vector.tensor_tensor(out=frac[:], in0=v[:],
                                                in1=f[:], op=OP.subtract)
                        return f, frac

                    py, ly = floorf(ys, "fy")   # py = floor(y)+1 (pad baked)
                    px, lx = floorf(xs_, "fx")

                    pyc = sm("pyc")
                    nc.vector.tensor_scalar(out=pyc[:], in0=py[:], scalar1=0.0,
                                            scalar2=64.0, op0=OP.max, op1=OP.min)
                    pxc = sm("pxc")
                    nc.vector.tensor_scalar(out=pxc[:], in0=px[:], scalar1=0.0,
                                            scalar2=64.0, op0=OP.max, op1=OP.min)
                    vy = sm("vy")
                    nc.vector.tensor_tensor(out=vy[:], in0=pyc[:], in1=py[:],
                                            op=OP.is_equal)
                    vx = sm("vx")
                    nc.vector.tensor_tensor(out=vx[:], in0=pxc[:], in1=px[:],
                                            op=OP.is_equal)
                    vv = sm("vv")
                    nc.vector.tensor_tensor(out=vv[:], in0=vy[:], in1=vx[:],
                                            op=OP.mult)

                    # mask = sigmoid(logits) * validity * dequant scale
                    mk = sm("mk")
                    nc.scalar.activation(
                        out=v3(mk[:]),
                        in_=omt[:, :, 18:27].rearrange("p c4 k -> p k c4"),
                        func=AF.Sigmoid)
                    mv = sm("mv")
                    nc.vector.tensor_tensor(out=mv[:], in0=mk[:], in1=vv[:],
                                            op=OP.mult)
                    nc.vector.scalar_tensor_tensor(
                        out=mv[:], in0=mv[:], scalar=ss[:, 0:1], in1=mv[:],
                        op0=OP.mult, op1=OP.bypass)

                    # corner weights: cw[a][b] = wy_a * wx_b * mv
                    wx1m = sm("wx1m")
                    nc.vector.tensor_tensor(out=wx1m[:], in0=lx[:], in1=mv[:],
                                            op=OP.mult)
                    wx0m = sm("wx0m")
                    nc.vector.tensor_tensor(out=wx0m[:], in0=mv[:], in1=wx1m[:],
                                            op=OP.subtract)
                    wy0 = sm("wy0")
                    nc.vector.tensor_scalar(out=wy0[:], in0=ly[:], scalar1=-1.0,
                                            scalar2=1.0, op0=OP.mult, op1=OP.add)
                    cw = {}
                    for (a, wya) in ((0, wy0), (1, ly)):
                        for (b, wxb) in ((0, wx0m), (1, wx1m)):
                            t = sm(f"cw{a}{b}")
                            nc.vector.tensor_tensor(out=t[:], in0=wya[:],
                                                    in1=wxb[:], op=OP.mult)
                            cw[(a, b)] = t

                    # ---- gather index: row = pyc*66 + pxc (int32) ----
                    tfi = sm("tfi")
                    nc.vector.tensor_scalar(out=tfi[:], in0=pyc[:],
                                            scalar1=float(NT), scalar2=None,
                                            op0=OP.mult)
                    nc.vector.tensor_tensor(out=tfi[:], in0=tfi[:], in1=pxc[:],
                                            op=OP.add)
                    idxi = coordp.tile([128, 36], I32, tag="idxi", name="idxi")
                    nc.vector.tensor_copy(out=idxi[:], in_=tfi[:])

                    if debug:
                        for nm, t in [("ys", ys), ("xs_", xs_), ("py", py),
                                      ("px", px), ("ly", ly), ("lx", lx),
                                      ("mv", mv), ("tfi", tfi)]:
                            dt_ = nc.dram_tensor(f"dbg_{nm}_{cq}", (128, 36),
                                                 F32, kind="ExternalOutput")
                            nc.sync.dma_start(out=dt_[:], in_=t[:])
                        for (a, b), t in cw.items():
                            dt_ = nc.dram_tensor(f"dbg_cw{a}{b}_{cq}",
                                                 (128, 36), F32,
                                                 kind="ExternalOutput")
                            nc.sync.dma_start(out=dt_[:], in_=t[:])

                    return cw, idxi

            def body(cq, cw, idxi):
                    # ---- per tap: 4 gathers + corner MAC + transpose ----
                    samp = {}
                    for k in range(K2):
                        acc = accp.tile([128, 4, C], BF16, tag="acc",
                                        name="acc")
                        for c4 in range(4):
                            col = k * 4 + c4
                            g = gbufp.tile([128, 4 * C], I8, tag="g", name="g")
                            nc.gpsimd.indirect_dma_start(
                                out=g[:], out_offset=None, in_=xq_t[:],
                                in_offset=bass.IndirectOffsetOnAxis(
                                    ap=idxi[:, col:col + 1], axis=0),
                            )
                            gs = [g[:, j * C:(j + 1) * C] for j in range(4)]
                            # corner order in row: (y0x0),(y0x1),(y1x0),(y1x1)
                            nc.scalar.activation(
                                out=acc[:, c4, :], in_=gs[0], func=AF.Copy,
                                scale=cw[(0, 0)][:, col:col + 1])
                            if col % 3 != 0:
                                # corner01 as ACT-mult + DVE add (load balance)
                                tmp = tmpp.tile([128, C], BF16, tag="tmp",
                                                name="tmp")
                                nc.scalar.activation(
                                    out=tmp[:], in_=gs[1], func=AF.Copy,
                                    scale=cw[(0, 1)][:, col:col + 1])
                                nc.vector.tensor_tensor(
                                    out=acc[:, c4, :], in0=acc[:, c4, :],
                                    in1=tmp[:], op=OP.add)
                            else:
                                nc.vector.scalar_tensor_tensor(
                                    out=acc[:, c4, :], in0=gs[1],
                                    scalar=cw[(0, 1)][:, col:col + 1],
                                    in1=acc[:, c4, :], op0=OP.mult, op1=OP.add)
                            nc.vector.scalar_tensor_tensor(
                                out=acc[:, c4, :], in0=gs[2],
                                scalar=cw[(1, 0)][:, col:col + 1],
                                in1=acc[:, c4, :], op0=OP.mult, op1=OP.add)
                            nc.vector.scalar_tensor_tensor(
                                out=acc[:, c4, :], in0=gs[3],
                                scalar=cw[(1, 1)][:, col:col + 1],
                                in1=acc[:, c4, :], op0=OP.mult, op1=OP.add)
                        # transpose [pos, ch] -> [ch, pos]
                        ps = ps_s.tile([128, 2, 4, 128], BF16, tag="ps",
                                       name="ps")
                        for ct in range(2):
                            for c4 in range(4):
                                nc.tensor.transpose(
                                    out=ps[:, ct, c4, :],
                                    in_=acc[:, c4, ct * 128:(ct + 1) * 128],
                                    identity=idnb[:],
                                )
                        st = sampp.tile([128, 2, 4, 128], BF16,
                                        tag=f"samp{k}", name=f"samp{k}")
                        nc.scalar.activation(
                            out=st[:].rearrange("p a b c -> p (a b c)"),
                            in_=ps[:].rearrange("p a b c -> p (a b c)"),
                            func=AF.Copy)
                        samp[k] = st

                    # ---- big matmul: out[256, 512] for this chunk ----
                    for m in range(2):
                        po = ps_out.tile([128, CQP], F32, tag="po", name="po")
                        for k in range(K2):
                            for ct in range(2):
                                nc.tensor.matmul(
                                    po[:],
                                    lhsT=w2_sb[k * 2 + ct][:, m * 128:
                                                           (m + 1) * 128],
                                    rhs=samp[k][:, ct].rearrange(
                                        "p a b -> p (a b)"),
                                    start=(k == 0 and ct == 0),
                                    stop=(k == K2 - 1 and ct == 1),
                                )
                        osb = outp.tile([128, CQP], F32, tag="osb", name="osb")
                        nc.scalar.activation(out=osb[:], in_=po[:],
                                             func=AF.Copy)
                        nc.sync.dma_start(
                            out=out_t[m * 128:(m + 1) * 128,
                                      cq * CQP:(cq + 1) * CQP],
                            in_=osb[:],
                        )

            # software pipeline: head runs one chunk ahead of body so the
            # gather descriptor-gen stream on Pool never waits for coords
            for rep in range(reps):
                hd = {0: head(0)}
                for cq in range(NCQ):
                    if cq + 1 < NCQ:
                        hd[cq + 1] = head(cq + 1)
                    body(cq, *hd.pop(cq))

    nc.compile()
    return nc


def host_inputs(x, w_off, b_off, w_conv):
    """Build the 8 per-core input maps (numpy only, layout prep)."""
    import ml_dtypes
    x = np.asarray(x, np.float32)
    w_off = np.asarray(w_off, np.float32)
    b_off = np.asarray(b_off, np.float32)
    w_conv = np.asarray(w_conv, np.float32)

    xp = np.zeros((B, C, 66, 66), np.float32)
    xp[:, :, 1:65, 1:65] = x
    wof = np.ascontiguousarray(
        w_off.reshape(27, C, K2).transpose(1, 2, 0)).reshape(C, K2 * 27)
    w2 = np.ascontiguousarray(
        w_conv.reshape(O, C, K2).transpose(2, 1, 0)).reshape(K2 * C, O)
    bof = b_off.reshape(27, 1).astype(np.float32)
    idnb = np.eye(128, dtype=ml_dtypes.bfloat16)
    idn27 = np.eye(27, dtype=np.float32)

    # int8 gather tables, one per batch image, on a 67x67 construction pad
    xq_b = []
    ss_b = []
    for b in range(B):
        s = float(np.abs(x[b]).max()) / 127.0
        q67 = np.zeros((67, 67, C), np.int8)
        q67[1:65, 1:65] = np.clip(
            np.rint(x[b].transpose(1, 2, 0) / s), -127, 127).astype(np.int8)
        xq = np.concatenate(
            [q67[:66, :66, None], q67[:66, 1:67, None],
             q67[1:67, :66, None], q67[1:67, 1:67, None]],
            axis=2).reshape(TROWS, 4 * C)
        xq_b.append(np.ascontiguousarray(xq))
        ss_b.append(np.full((128, 1), s, np.float32))

    in_maps = []
    p = np.arange(128)
    k = np.arange(K2)
    c4 = np.arange(4)
    cqv = np.arange(NCQ)
    for core in range(8):
        b, sh = core // 2, core % 2
        xs = np.ascontiguousarray(
            xp[b, :, sh * 32: sh * 32 + 34, :]).reshape(C, 34 * 66)
        # pos = cq*512 + c4*128 + p ; row = pos//64 ; col = pos%64
        pos = (cqv[:, None, None, None] * 512 + c4[None, None, :, None] * 128
               + p[None, None, None, :])                    # [cq, 1, c4, p]
        pos = np.broadcast_to(pos, (NCQ, K2, 4, 128))
        row = sh * 32 + pos // 64
        colw = pos % 64
        # padded-grid base incl. +1 pad offset: floor(y)+1 = floor(y + base+1)
        byt = (row + (k[None, :, None, None] // 3)).astype(np.float32)
        bxt = (colw + (k[None, :, None, None] % 3)).astype(np.float32)
        byt = byt.transpose(3, 0, 1, 2).reshape(128, NCQ * 36)
        bxt = bxt.transpose(3, 0, 1, 2).reshape(128, NCQ * 36)
        in_maps.append({
            "xs": xs, "wof": wof, "bof": bof, "w2": w2, "xq": xq_b[b],
            "byt": np.ascontiguousarray(byt),
            "bxt": np.ascontiguousarray(bxt),
            "ss": ss_b[b], "idnb": idnb, "idn27": idn27,
        })
    return in_maps


_NC = None


def kernel(x, w_off, b_off, w_conv):
    global _NC
    if _NC is None:
        _NC = build_program()
    in_maps = host_inputs(x, w_off, b_off, w_conv)
    res = run_bass_kernel_spmd(_NC, in_maps, core_ids=list(range(8)))
    out = np.empty((B, O, H, W), np.float32)
    for core in range(8):
        b, sh = core // 2, core % 2
        out[b, :, sh * 32:(sh + 1) * 32, :] = res.results[core]["out"].reshape(
            O, 32, 64)
    return out
